# revision 5
# baseline (speedup 1.0000x reference)
"""Trainium2 Bass kernel: 4096x4096 fp32 'valid' cross-correlation with a 15x15
kernel, plus scalar bias.

Strategy
--------
- Shard output columns (W) across 8 NeuronCores: each core computes all 4082
  output rows for a 512-column stripe (core 7's tail trimmed on host). Each
  core's input stripe is its 512 columns plus a 14-column halo, gathered on
  the host -- no device-to-device communication.
- Per core the conv is 1920 K=64, M=32, N=512 bf16 matmuls: for each kernel
  column dj and each 32-row output block, one banded-Toeplitz stationary
  [64, 32] (T[q, m] = weight[q - m, dj]) contracts a 64-row input window
  against 32 output rows, with the W-shift dj absorbed as a free-dim offset.
- The PE array is addressed as 32x32 sub-tiles via tile_position: output
  blocks are spread over 4 PSUM column groups and 2 row-strip pairs, so 8
  dominoes (16 sub-arrays) execute concurrently. Every block is a single
  64-aligned K=64 matmul because SBUF holds TWO copies of the image: one
  as-is and one shifted down by 32 rows (odd blocks read the shifted copy).
  This packs the 15-wide Toeplitz band ~2.25x denser per streamed cycle than
  a full-width 128x114 formulation.
- Inputs/weights are bf16 (DMA halved, matmul still 1 cycle/row), PSUM
  accumulates fp32, outputs stored bf16 and upcast on host (rel err ~3e-3,
  well under the 2e-2 gate).
"""

import numpy as np

H, W = 4096, 4096
KH, KW = 15, 15
HO, WO = H - KH + 1, W - KW + 1  # 4082, 4082
NCORES = 8
C = 512              # output cols per core
CIN = C + KW - 1     # input cols per core stripe (with halo) = 526
NB = H // 128        # 32 free-dim row-blocks in the SBUF-resident image
NSG = 8              # supergroups of 16 output row-blocks (32 rows each)

_CACHE = {}


def _build_nc(reps: int = 1, loop: bool = False):
    """Build the kernel. loop=True wraps the rep body in a hardware For_i
    whose trip count comes from the int32 input "nreps" -- one executable
    serves any rep count, enabling exact self-relative timing."""
    import concourse.bacc as bacc
    import concourse.mybir as mybir
    from concourse.tile import TileContext

    f32 = mybir.dt.float32
    bf16 = mybir.dt.bfloat16

    nc = bacc.Bacc("TRN2", debug=False, num_devices=NCORES)
    xs_d = nc.dram_tensor("xs", [128, NB, CIN], bf16, kind="ExternalInput")
    xs2_d = nc.dram_tensor("xs2", [128, NB, CIN], bf16, kind="ExternalInput")
    wT_d = nc.dram_tensor("wT", [128, KW, 32], bf16, kind="ExternalInput")
    bias_d = nc.dram_tensor("bias", [1, 1], f32, kind="ExternalInput")
    if loop:
        nreps_d = nc.dram_tensor("nreps", [1, 1], mybir.dt.int32,
                                 kind="ExternalInput")
    ys_d = nc.dram_tensor("ys", [NSG * 4, 128, C], bf16, kind="ExternalOutput")

    with TileContext(nc) as tc:
        with (
            tc.tile_pool(name="xp", bufs=2) as xp,
            tc.tile_pool(name="wp", bufs=1) as wp,
            tc.tile_pool(name="op", bufs=2) as op,
            tc.tile_pool(name="pp", bufs=2, space="PSUM") as pp,
        ):
            # Band-Toeplitz stationary stack + bias
            w_t = wp.tile([128, KW, 32], bf16)
            nc.sync.dma_start(w_t[:, :, :], wT_d[:, :, :])
            bias_t = wp.tile([1, 1], f32)
            nc.sync.dma_start(bias_t[:, :], bias_d[:, :])
            bias_bc = wp.tile([128, 1], f32)
            nc.gpsimd.partition_broadcast(bias_bc[:, :], bias_t[:, :])

            def rep_body():
                xt = xp.tile([128, NB, CIN], bf16, name="xt")
                xt2 = xp.tile([128, NB, CIN], bf16, name="xt2")
                # First supergroup's blocks land as small low-latency DMAs;
                # the rest as larger chunks spread over the DMA queues.
                for i in range(0, 4):
                    nc.sync.dma_start(xt[:, i, :], xs_d[:, i, :])
                    nc.sync.dma_start(xt2[:, i, :], xs2_d[:, i, :])
                for i in range(4, NB, 4):
                    nc.sync.dma_start(xt[:, i:i + 4, :], xs_d[:, i:i + 4, :])
                    nc.sync.dma_start(xt2[:, i:i + 4, :], xs2_d[:, i:i + 4, :])

                for sg in range(NSG):
                    pss = [pp.tile([128, C], f32, name=f"ps{t}") for t in range(4)]
                    for dj in range(KW):
                        st, sp = dj == 0, dj == KW - 1
                        # Round 1: even blocks (t=0,2) from xt; round 2: odd
                        # blocks (t=1,3) from the 32-row-shifted copy xt2.
                        for t in (0, 2, 1, 3):
                            src = xt if t in (0, 2) else xt2
                            lo = 0 if t in (0, 1) else 64
                            for c in range(4):
                                b = 4 * sg + c
                                nc.tensor.matmul(
                                    pss[t][32 * c:32 * c + 32, :],
                                    w_t[lo:lo + 64, dj, :],
                                    src[lo:lo + 64, b, dj:dj + C],
                                    start=st,
                                    stop=sp,
                                    tile_position=(lo, 32 * c),
                                    skip_group_check=True,
                                )
                    for t in range(4):
                        o = op.tile([128, C], bf16, name=f"o{t}")
                        nc.vector.tensor_scalar_add(
                            o[:, :], pss[t][:, :], bias_bc[:, 0:1]
                        )
                        if sg == NSG - 1:
                            # split the tail DMAs so the drain isn't gated
                            # by a single ~27 GB/s queue
                            nc.sync.dma_start(
                                ys_d[4 * sg + t, :, 0:C // 2], o[:, 0:C // 2])
                            nc.sync.dma_start(
                                ys_d[4 * sg + t, :, C // 2:C], o[:, C // 2:C])
                        else:
                            nc.sync.dma_start(ys_d[4 * sg + t, :, :], o[:, :])

            if loop:
                nr_t = wp.tile([1, 1], mybir.dt.int32)
                nc.sync.dma_start(nr_t[:, :], nreps_d[:, :])
                regs = nc.alloc_registers("nreps")
                nc.regs_load(regs, nr_t[0:1, 0:1])
                end = nc.snap(regs, donate=True, min_val=1, max_val=100000)
                with tc.For_i(0, end):
                    rep_body()
            else:
                for _rep in range(reps):
                    rep_body()

    nc.compile()
    return nc


def prepare_in_maps(x: np.ndarray, weight: np.ndarray, bias: np.ndarray):
    """Host-side shard + layout: per-core input maps for _build_nc's tensors."""
    from ml_dtypes import bfloat16

    x = np.ascontiguousarray(x, dtype=np.float32)
    w = np.asarray(weight, dtype=np.float32)
    b = np.asarray(bias, dtype=np.float32).reshape(-1)[:1]

    WPAD = NCORES * C + KW - 1
    x_pad = np.zeros((H + 32, WPAD), dtype=np.float32)
    x_pad[:H, :W] = x

    # wT[p = 32*s + q, dj, m] = w[q + (32 if s odd else 0) - m, dj]
    wT = np.zeros((128, KW, 32), dtype=np.float32)
    for s in range(4):
        off = 32 if s % 2 else 0
        for q in range(32):
            for m in range(32):
                di = q + off - m
                if 0 <= di < KH:
                    wT[32 * s + q, :, m] = w[di, :]
    wT16 = np.ascontiguousarray(wT.astype(bfloat16))
    bias_in = b.reshape(1, 1)

    in_maps = []
    for cidx in range(NCORES):
        st = x_pad[:H, cidx * C:cidx * C + CIN]
        st2 = x_pad[32:32 + H, cidx * C:cidx * C + CIN]
        xs = np.ascontiguousarray(
            st.reshape(NB, 128, CIN).transpose(1, 0, 2).astype(bfloat16))
        xs2 = np.ascontiguousarray(
            st2.reshape(NB, 128, CIN).transpose(1, 0, 2).astype(bfloat16))
        in_maps.append({"xs": xs, "xs2": xs2, "wT": wT16, "bias": bias_in})
    return in_maps


def kernel(x: np.ndarray, weight: np.ndarray, bias: np.ndarray) -> np.ndarray:
    from concourse.bass_utils import run_bass_kernel_spmd

    if "nc" not in _CACHE:
        _CACHE["nc"] = _build_nc()
    nc = _CACHE["nc"]

    in_maps = prepare_in_maps(x, weight, bias)
    res = run_bass_kernel_spmd(nc, in_maps, core_ids=list(range(NCORES)))

    out = np.empty((HO, WO), dtype=np.float32)
    for cidx in range(NCORES):
        y = np.asarray(res.results[cidx]["ys"])          # [32, 128, C] bf16
        y = (y.reshape(NSG, 4, 4, 32, C)
              .transpose(0, 2, 1, 3, 4)
              .reshape(NSG * 512, C)[:HO]
              .astype(np.float32))
        c0 = cidx * C
        c1 = min(c0 + C, WO)
        out[:, c0:c1] = y[:, :c1 - c0]
    return out
